# revision 6
# baseline (speedup 1.0000x reference)
"""Trainium2 Bass kernel for nn_BPSpikingNet (3-layer LIF spiking net).

Strategy (data-parallel over batch, 8 NeuronCores, zero collectives):
  - Host pre-packs x and all weights into fp8(e4m3) DoubleRow pair layouts;
    weights scaled by 64 (pow2, exact) to stay in fp8 normal range, unscaled
    via the ACT psum->sbuf copy's scale parameter. fp32 PSUM accumulation.
  - Matmuls run in MatmulPerfMode.DoubleRow: contraction 256 per pass
    (2x MACs/cycle vs fp16) -> L0: 3 passes, L1/L2: 4 passes per o-chunk.
  - Per core: stream T=100 in chunks of 10 steps. The three layers' LIF
    scans run as ONE concatenated DVE scan over [128, 544] fp16 state
    (L1 lags L0 by 2 waves, L2 by 3), 3 ops/step:
      u = w + z             [tensor_tensor]
      q = (u < 1) * 0.5     [tensor_scalar, fp16 out -> 2x DVE mode]
      w = u * q             [tensor_tensor]
  - Spikes are re-coded to fp8 for matmul consumption by a per-wave
    gpsimd cast-DMA (values {0,0.5} are exact in fp8); L1/L2 matmuls read
    the fp8 buffer through a strided (pair, t, batch) access pattern.
  - Output: q2 in [20, T, 32] per core; host maps s = 1 - 2q and gathers.
"""
import sys

import numpy as np

sys.path.insert(0, "/opt/trn_rl_repo")

import concourse.bass as bass
import concourse.tile as tile
from concourse import bacc, mybir
from concourse.bass_utils import run_bass_kernel_spmd
import ml_dtypes


def _install_ntff_shim():
    """Provide antenv.axon_hooks (missing in the trimmed image) so that
    trace=True NTFF profiling works when requested via BASS_TRACE."""
    try:
        import antenv.axon_hooks  # noqa: F401
        return
    except ImportError:
        pass
    try:
        import types

        import antenv

        mod = types.ModuleType("antenv.axon_hooks")
        holder = {"h": None}
        mod.set_axon_ntff_profile_hook = lambda h: holder.__setitem__("h", h)
        mod.get_axon_ntff_profile_hook = lambda: holder["h"]
        sys.modules["antenv.axon_hooks"] = mod
        antenv.axon_hooks = mod
        try:
            from trn_agent_boot.trn_boot import _ntff_profile_via_ctypes

            h = _ntff_profile_via_ctypes("/opt/axon/libaxon_pjrt.so")
            if h is not None:
                mod.set_axon_ntff_profile_hook(h)
        except Exception:
            pass
    except Exception:
        pass


_install_ntff_shim()

F32 = mybir.dt.float32
F16 = mybir.dt.float16
F8 = mybir.dt.float8e4
E4 = ml_dtypes.float8_e4m3
AL = mybir.AluOpType
AF = mybir.ActivationFunctionType
DR = mybir.MatmulPerfMode.DoubleRow

T, B, FIN, HID, CLS = 100, 256, 700, 1024, 20
NCORES = 8
BC = B // NCORES            # 32 batch rows per core
TC = 10                     # timesteps per chunk
NCHUNK = T // TC            # 10
NWAVE = NCHUNK + 4          # L1 lags L0 by 2 waves, L2 lags L1 by 2
P0 = 3                      # DoubleRow passes for layer 0 (K=768)
P1 = 4                      # DoubleRow passes for layers 1/2 (K=1024)
SEC = 544                   # 256 (L0 out) + 256 (L1 out) + 32 (L2 out)
WSC = 64.0                  # fp8 weight scale (pow2); ACT copies scale 1/64

_CACHE = {}
LAST_RESULT = None


def _active_window(w):
    """Column window [lo, hi) of the concat state active at wave w."""
    lo = 0 if w <= NCHUNK - 1 else (256 if w <= NCHUNK + 1 else 512)
    hi = 256 if w < 2 else (512 if w < 4 else SEC)
    return lo, hi


def _build():
    nc = bacc.Bacc(None, target_bir_lowering=False)
    XT = nc.declare_dram_parameter("XT", [P0, 128, 2, T, BC], F8, isOutput=False)
    W0T = nc.declare_dram_parameter("W0T", [P0, 128, 2, HID], F8, isOutput=False)
    W1T = nc.declare_dram_parameter("W1T", [P1, 128, 2, HID], F8, isOutput=False)
    W2T = nc.declare_dram_parameter("W2T", [P1, 128, 2, 128], F8, isOutput=False)
    BIAS = nc.declare_dram_parameter("BIAS", [128, 17], F32, isOutput=False)
    QOUT = nc.declare_dram_parameter("QOUT", [CLS, T, BC], F16, isOutput=True)

    with tile.TileContext(nc) as tc:
        with (
            tc.tile_pool(name="const", bufs=1) as cp,
            tc.tile_pool(name="zp", bufs=5) as zp,
            tc.tile_pool(name="sp", bufs=5) as sp,
            tc.tile_pool(name="qp", bufs=4) as qp,
            tc.tile_pool(name="up", bufs=4) as up,
            tc.tile_pool(name="pp", bufs=6, space=bass.MemorySpace.PSUM) as pp,
        ):
            xt = [cp.tile([128, 2, T, BC], F8, tag=f"xt{k}", name=f"xt{k}")
                  for k in range(P0)]
            w0 = [cp.tile([128, 2, HID], F8, tag=f"w0_{k}", name=f"w0_{k}")
                  for k in range(P0)]
            w1 = [cp.tile([128, 2, HID], F8, tag=f"w1_{k}", name=f"w1_{k}")
                  for k in range(P1)]
            w2 = [cp.tile([128, 2, 128], F8, tag=f"w2_{k}", name=f"w2_{k}")
                  for k in range(P1)]
            bias = cp.tile([128, 17], F32, tag="bias")
            wst = cp.tile([128, SEC], F16, tag="wst")
            outq = cp.tile([CLS, T, BC], F16, tag="outq")

            nc.scalar.dma_start(bias[:], BIAS[:])
            for k in range(P0):
                nc.sync.dma_start(w0[k][:], W0T[k])
                nc.sync.dma_start(xt[k][:, :, 0:TC, :], XT[k][:, :, 0:TC, :])
            for k in range(P0):
                nc.sync.dma_start(xt[k][:, :, TC:T, :], XT[k][:, :, TC:T, :])
            for k in range(P1):
                nc.sync.dma_start(w1[k][:], W1T[k])
                nc.sync.dma_start(w2[k][:], W2T[k])
            nc.gpsimd.memset(wst[:], 0.0)
            # warm the ACT activation-table during the DMA head so the first
            # psum->sbuf copy doesn't pay the ~2.7us table load
            warm = cp.tile([128, 1], F32, tag="warm")
            nc.vector.memset(warm[:], 0.0)
            nc.scalar.activation(warm[:], warm[:], AF.Identity,
                                 bias=bias[:, 0:1], scale=1.0)

            prev_q8 = [None, None]  # Q8(w-1), Q8(w-2) spike buffers (fp8)
            pend_act = None  # (q8f, sw) of prev wave: ACT L0-convert, delayed
                             # so it doesn't head-of-line block this wave's copies

            ISC = 1.0 / WSC

            for w in range(NWAVE):
                zw = zp.tile([128, TC, SEC], F16, tag="z", name=f"z{w}")

                # --- fill Z(w): tensor engine + scalar-engine copies ---
                if w <= NCHUNK - 1:  # L0 chunk w
                    halves = 2 if w == 0 else 1
                    half = TC // halves
                    for hv in range(halves):
                        tsl = slice(hv * half, (hv + 1) * half)
                        for o in range(8):
                            ps = pp.tile([128, half, BC], F32, tag="ps", name="ps")
                            for k in range(P0):
                                nc.tensor.matmul(
                                    ps[:],
                                    w0[k][:, :, o * 128:(o + 1) * 128],
                                    xt[k][:, :, w * TC + hv * half:w * TC + (hv + 1) * half, :],
                                    start=(k == 0), stop=(k == P0 - 1),
                                    perf_mode=DR,
                                )
                            nc.scalar.activation(
                                zw[:, tsl, o * 32:(o + 1) * 32], ps[:],
                                AF.Identity, bias=bias[:, o:o + 1], scale=ISC,
                            )
                if 2 <= w <= NCHUNK + 1:  # L1 chunk w-2, consumes Q8(w-2) L0 part
                    q8in = prev_q8[1]
                    halves = 2 if w == NCHUNK + 1 else 1
                    half = TC // halves
                    for hv in range(halves):
                        tsl = slice(hv * half, (hv + 1) * half)
                        for o in range(8):
                            ps = pp.tile([128, half, BC], F32, tag="ps", name="ps")
                            for k in range(P1):
                                nc.tensor.matmul(
                                    ps[:],
                                    w1[k][:, :, o * 128:(o + 1) * 128],
                                    q8in[:, tsl, k, :, :].rearrange(
                                        "p t i b -> p i t b"),
                                    start=(k == 0), stop=(k == P1 - 1),
                                    perf_mode=DR,
                                )
                            nc.scalar.activation(
                                zw[:, tsl, 256 + o * 32:256 + (o + 1) * 32], ps[:],
                                AF.Identity, bias=bias[:, 8 + o:9 + o], scale=ISC,
                            )
                if 4 <= w <= NCHUNK + 3:  # L2 chunk w-4, consumes Q8(w-2) L1 part
                    q8in = prev_q8[1]
                    nparts = 2
                    half = TC // nparts
                    for hv in range(nparts):
                        tsl = slice(hv * half, (hv + 1) * half)
                        ps = pp.tile([128, half, BC], F32, tag="ps2", name="ps2", bufs=2)
                        for k in range(P1):
                            nc.tensor.matmul(
                                ps[:],
                                w2[k][:],
                                q8in[:, tsl, 4 + k, :, :].rearrange(
                                    "p t i b -> p i t b"),
                                start=(k == 0), stop=(k == P1 - 1),
                                perf_mode=DR,
                            )
                        nc.scalar.activation(
                            zw[:, tsl, 512:SEC], ps[:, :, 0:32],
                            AF.Identity, bias=bias[:, 16:17], scale=ISC,
                        )

                # ACT L0-section convert of the PREVIOUS wave (after this
                # wave's psum->sbuf copies in ACT program order, so those
                # copies are not blocked behind the previous wave's scan)
                if pend_act is not None:
                    pq8f, psw = pend_act
                    nc.scalar.copy(pq8f[:, :, 0:256], psw[:, :, 0:256])
                    pend_act = None

                # --- scan wave w: 10 LIF steps over the active window ---
                lo, hi = _active_window(w)
                sw = sp.tile([128, TC, SEC], F16, tag="s", name=f"s{w}")
                for t in range(TC):
                    # state wst = 0.5 * v_post; u = v_pre; q = 0.5*(u<1)
                    u = up.tile([128, SEC], F16, tag="u", name="u")
                    nc.vector.tensor_tensor(
                        u[:, lo:hi], wst[:, lo:hi], zw[:, t, lo:hi], op=AL.add,
                    )
                    nc.vector.tensor_scalar(
                        sw[:, t, lo:hi], u[:, lo:hi], 1.0, 0.5,
                        op0=AL.is_lt, op1=AL.mult,
                    )
                    nc.vector.tensor_tensor(
                        wst[:, lo:hi], u[:, lo:hi], sw[:, t, lo:hi], op=AL.mult,
                    )

                # re-code spikes to fp8 for matmul consumption (exact: {0,0.5})
                # split across ACT (L0 half, delayed one wave) and DVE (L1
                # half, chain-resident right after the scan) to balance load
                q8 = qp.tile([128, TC, 8, 2, 32], F8, tag="q8", name=f"q8{w}")
                q8f = q8[:].rearrange("p t k i b -> p t (k i b)")
                if w <= NCHUNK - 1:      # L0 section feeds L1-mm at w+2
                    pend_act = (q8f, sw)
                if 2 <= w <= NCHUNK + 1:  # L1 section feeds L2-mm at w+1
                    nc.vector.tensor_scalar(
                        q8f[:, :, 256:512], sw[:, :, 256:512], 0.0, None,
                        op0=AL.add,
                    )

                if w >= 4:  # collect L2 spikes (chunk w-4)
                    nc.scalar.copy(
                        outq[:, (w - 4) * TC:(w - 3) * TC, :],
                        sw[0:CLS, :, 512:SEC],
                    )

                prev_q8 = [q8, prev_q8[0]]

            nc.sync.dma_start(QOUT[:], outq[:])

    nc.compile()
    return nc


def _get_nc():
    if "nc" not in _CACHE:
        _CACHE["nc"] = _build()
    return _CACHE["nc"]


def _get_runner():
    """Build (once) a cached jitted SPMD executable over the 8 cores."""
    if "runner" in _CACHE:
        return _CACHE["runner"]
    import jax
    from jax.sharding import Mesh, PartitionSpec
    from jax.experimental.shard_map import shard_map
    from concourse import bass2jax

    nc = _get_nc()
    bass2jax.install_neuronx_cc_hook()
    partition_name = (
        nc.partition_id_tensor.name if nc.partition_id_tensor else None
    )
    in_names, out_names, out_avals, zero_shapes = [], [], [], []
    for alloc in nc.m.functions[0].allocations:
        if not isinstance(alloc, mybir.MemoryLocationSet):
            continue
        name = alloc.memorylocations[0].name
        if alloc.kind == "ExternalInput":
            if name != partition_name:
                in_names.append(name)
        elif alloc.kind == "ExternalOutput":
            shape = tuple(alloc.tensor_shape)
            dtype = mybir.dt.np(alloc.dtype)
            out_names.append(name)
            out_avals.append(jax.core.ShapedArray(shape, dtype))
            zero_shapes.append((shape, dtype))
    n_params = len(in_names)
    all_in = in_names + out_names
    if partition_name is not None:
        all_in = all_in + [partition_name]

    def _body(*args):
        operands = list(args)
        if partition_name is not None:
            operands.append(bass2jax.partition_id_tensor())
        outs = bass2jax._bass_exec_p.bind(
            *operands,
            out_avals=tuple(out_avals),
            in_names=tuple(all_in),
            out_names=tuple(out_names),
            lowering_input_output_aliases=(),
            sim_require_finite=True,
            sim_require_nnan=True,
            nc=nc,
        )
        return tuple(outs)

    devices = jax.devices()[:NCORES]
    mesh = Mesh(np.asarray(devices), ("core",))
    donate = tuple(range(n_params, n_params + len(out_names)))
    sharded = jax.jit(
        shard_map(
            _body, mesh=mesh,
            in_specs=(PartitionSpec("core"),) * (n_params + len(out_names)),
            out_specs=(PartitionSpec("core"),) * len(out_names),
            check_rep=False,
        ),
        donate_argnums=donate, keep_unused=True,
    )

    def run(in_maps):
        concat_in = [
            np.concatenate([np.asarray(m[nm]) for m in in_maps], axis=0)
            for nm in in_names
        ]
        concat_zeros = [
            np.zeros((NCORES * sh[0], *sh[1:]), dt) for sh, dt in zero_shapes
        ]
        out_arrs = sharded(*concat_in, *concat_zeros)
        return [
            {
                nm: np.asarray(out_arrs[i]).reshape(NCORES, *out_avals[i].shape)[c]
                for i, nm in enumerate(out_names)
            }
            for c in range(NCORES)
        ]

    _CACHE["runner"] = run
    return run


def _to_f8(a):
    return np.clip(np.asarray(a, np.float32), -240.0, 240.0).astype(E4)


def _pack_dr_weights(wt, passes):
    """wt: [K, M] fp32 (K = passes*256) -> [passes, 128, 2, M] fp8."""
    K, M = wt.shape
    assert K == passes * 256
    return _to_f8(wt.reshape(passes, 2, 128, M).transpose(0, 2, 1, 3))


def kernel(x_tbf, W0, b0, W1, b1, W2, b2):
    global LAST_RESULT
    import os

    x = np.asarray(x_tbf, np.float32)
    W0 = np.asarray(W0, np.float32)
    W1 = np.asarray(W1, np.float32)
    W2 = np.asarray(W2, np.float32)
    b0 = np.asarray(b0, np.float32)
    b1 = np.asarray(b1, np.float32)
    b2 = np.asarray(b2, np.float32)

    # weights: fold the 0.5 (leak) scale and the q-code correction (s = 1-2q)
    # plus the fp8 range scale WSC (unscaled in the ACT psum->sbuf copy).
    w0t = np.zeros((P0 * 256, HID), np.float32)
    w0t[:FIN] = WSC * 0.5 * W0.T
    w1t = WSC * (-W1.T)                                   # [1024, 1024]
    w2t = np.zeros((HID, 128), np.float32)
    w2t[:, :CLS] = WSC * (-W2.T)

    w0t_r = _pack_dr_weights(w0t, P0)
    w1t_r = _pack_dr_weights(w1t, P1)
    w2t_r = _pack_dr_weights(w2t, P1)

    bias_arr = np.zeros((128, 17), np.float32)
    bias_arr[:, 0:8] = (0.5 * b0).reshape(8, 128).T
    b1e = 0.5 * (b1.astype(np.float64) + W1.astype(np.float64).sum(axis=1))
    bias_arr[:, 8:16] = b1e.astype(np.float32).reshape(8, 128).T
    b2e = 0.5 * (b2.astype(np.float64) + W2.astype(np.float64).sum(axis=1))
    bias_arr[:CLS, 16] = b2e.astype(np.float32)

    in_maps = []
    for c in range(NCORES):
        xs = x[:, c * BC:(c + 1) * BC, :]                 # [T, BC, FIN]
        xt = np.zeros((P0 * 256, T, BC), np.float32)
        xt[:FIN] = xs.transpose(2, 0, 1)
        xt_r = _to_f8(xt.reshape(P0, 2, 128, T, BC).transpose(0, 2, 1, 3, 4))
        in_maps.append({
            "XT": np.ascontiguousarray(xt_r),
            "W0T": w0t_r, "W1T": w1t_r, "W2T": w2t_r, "BIAS": bias_arr,
        })

    if os.environ.get("BASS_TRACE"):
        nc = _get_nc()
        LAST_RESULT = run_bass_kernel_spmd(
            nc, in_maps, list(range(NCORES)),
            trace=True,
            tmpdir=os.environ.get("BASS_TRACE_DIR"),
        )
        results = LAST_RESULT.results
    else:
        results = _get_runner()(in_maps)

    out = np.empty((T, B, CLS), np.float32)
    for c in range(NCORES):
        q = results[c]["QOUT"].astype(np.float32)  # [CLS, T, BC]
        out[:, c * BC:(c + 1) * BC, :] = (1.0 - 2.0 * q).transpose(1, 2, 0)
    return out


# revision 7
# speedup vs baseline: 1.1184x; 1.1184x over previous
"""Trainium2 Bass kernel for nn_BPSpikingNet (3-layer LIF spiking net).

Strategy (data-parallel over batch, 8 NeuronCores, zero collectives):
  - Host pre-packs x and all weights into fp8(e4m3) DoubleRow pair layouts;
    weights scaled by 64 (pow2, exact) to stay in fp8 normal range, unscaled
    via the ACT psum->sbuf copy's scale parameter. fp32 PSUM accumulation.
  - Matmuls run in MatmulPerfMode.DoubleRow: contraction 256 per pass
    (2x MACs/cycle vs fp16) -> L0: 3 passes, L1/L2: 4 passes per o-chunk.
  - Per core: stream T=100 in chunks of 10 steps. The three layers' LIF
    scans run as ONE concatenated DVE scan over [128, 544] fp16 state
    (L1 lags L0 by 2 waves, L2 by 3), 3 ops/step:
      u = w + z             [tensor_tensor]
      q = (u < 1) * 0.5     [tensor_scalar, fp16 out -> 2x DVE mode]
      w = u * q             [tensor_tensor]
  - Spikes are re-coded to fp8 for matmul consumption by a per-wave
    gpsimd cast-DMA (values {0,0.5} are exact in fp8); L1/L2 matmuls read
    the fp8 buffer through a strided (pair, t, batch) access pattern.
  - Output: q2 in [20, T, 32] per core; host maps s = 1 - 2q and gathers.
"""
import sys

import numpy as np

sys.path.insert(0, "/opt/trn_rl_repo")

import concourse.bass as bass
import concourse.tile as tile
from concourse import bacc, mybir
from concourse.bass_utils import run_bass_kernel_spmd
import ml_dtypes


def _install_ntff_shim():
    """Provide antenv.axon_hooks (missing in the trimmed image) so that
    trace=True NTFF profiling works when requested via BASS_TRACE."""
    try:
        import antenv.axon_hooks  # noqa: F401
        return
    except ImportError:
        pass
    try:
        import types

        import antenv

        mod = types.ModuleType("antenv.axon_hooks")
        holder = {"h": None}
        mod.set_axon_ntff_profile_hook = lambda h: holder.__setitem__("h", h)
        mod.get_axon_ntff_profile_hook = lambda: holder["h"]
        sys.modules["antenv.axon_hooks"] = mod
        antenv.axon_hooks = mod
        try:
            from trn_agent_boot.trn_boot import _ntff_profile_via_ctypes

            h = _ntff_profile_via_ctypes("/opt/axon/libaxon_pjrt.so")
            if h is not None:
                mod.set_axon_ntff_profile_hook(h)
        except Exception:
            pass
    except Exception:
        pass


_install_ntff_shim()

F32 = mybir.dt.float32
F16 = mybir.dt.float16
F8 = mybir.dt.float8e4
E4 = ml_dtypes.float8_e4m3
AL = mybir.AluOpType
AF = mybir.ActivationFunctionType
DR = mybir.MatmulPerfMode.DoubleRow

T, B, FIN, HID, CLS = 100, 256, 700, 1024, 20
NCORES = 8
BC = B // NCORES            # 32 batch rows per core
TC = 10                     # timesteps per chunk
NCHUNK = T // TC            # 10
NWAVE = NCHUNK + 3          # L1 lags L0 by 2 waves, L2 lags L1 by 1
P0 = 3                      # DoubleRow passes for layer 0 (K=768)
P1 = 4                      # DoubleRow passes for layers 1/2 (K=1024)
SEC = 544                   # 256 (L0 out) + 256 (L1 out) + 32 (L2 out)
WSC = 64.0                  # fp8 weight scale (pow2); ACT copies scale 1/64

_CACHE = {}
LAST_RESULT = None


def _active_window(w):
    """Column window [lo, hi) of the concat state active at wave w."""
    lo = 0 if w <= NCHUNK - 1 else (256 if w <= NCHUNK + 1 else 512)
    hi = 256 if w < 2 else (512 if w < 3 else SEC)
    return lo, hi


def _build():
    nc = bacc.Bacc(None, target_bir_lowering=False)
    XT = nc.declare_dram_parameter("XT", [P0, 128, 2, T, BC], F8, isOutput=False)
    W0T = nc.declare_dram_parameter("W0T", [P0, 128, 2, HID], F8, isOutput=False)
    W1T = nc.declare_dram_parameter("W1T", [P1, 128, 2, HID], F8, isOutput=False)
    W2T = nc.declare_dram_parameter("W2T", [P1, 128, 2, 128], F8, isOutput=False)
    BIAS = nc.declare_dram_parameter("BIAS", [128, 17], F32, isOutput=False)
    QOUT = nc.declare_dram_parameter("QOUT", [CLS, T, BC], F16, isOutput=True)

    with tile.TileContext(nc) as tc:
        with (
            tc.tile_pool(name="const", bufs=1) as cp,
            tc.tile_pool(name="zp", bufs=5) as zp,
            tc.tile_pool(name="sp", bufs=5) as sp,
            tc.tile_pool(name="qp", bufs=4) as qp,
            tc.tile_pool(name="up", bufs=4) as up,
            tc.tile_pool(name="pp", bufs=6, space=bass.MemorySpace.PSUM) as pp,
        ):
            xt = [cp.tile([128, 2, T, BC], F8, tag=f"xt{k}", name=f"xt{k}")
                  for k in range(P0)]
            w0 = [cp.tile([128, 2, HID], F8, tag=f"w0_{k}", name=f"w0_{k}")
                  for k in range(P0)]
            w1 = [cp.tile([128, 2, HID], F8, tag=f"w1_{k}", name=f"w1_{k}")
                  for k in range(P1)]
            w2 = [cp.tile([128, 2, 128], F8, tag=f"w2_{k}", name=f"w2_{k}")
                  for k in range(P1)]
            bias = cp.tile([128, 17], F32, tag="bias")
            wst = cp.tile([128, SEC], F16, tag="wst")
            outq = cp.tile([CLS, T, BC], F16, tag="outq")

            nc.scalar.dma_start(bias[:], BIAS[:])
            for k in range(P0):
                nc.sync.dma_start(w0[k][:], W0T[k])
                nc.sync.dma_start(xt[k][:, :, 0:TC, :], XT[k][:, :, 0:TC, :])
            for k in range(P0):
                nc.sync.dma_start(xt[k][:, :, TC:T, :], XT[k][:, :, TC:T, :])
            for k in range(P1):
                nc.sync.dma_start(w1[k][:], W1T[k])
                nc.sync.dma_start(w2[k][:], W2T[k])
            nc.gpsimd.memset(wst[:], 0.0)
            # warm the ACT activation-table during the DMA head so the first
            # psum->sbuf copy doesn't pay the ~2.7us table load
            warm = cp.tile([128, 1], F32, tag="warm")
            nc.vector.memset(warm[:], 0.0)
            nc.scalar.activation(warm[:], warm[:], AF.Identity,
                                 bias=bias[:, 0:1], scale=1.0)

            prev_q8 = [None, None]  # Q8(w-1), Q8(w-2) spike buffers (fp8)
            pend_act = None  # (q8f, sw) of prev wave: ACT L0-convert, delayed
                             # so it doesn't head-of-line block this wave's copies

            ISC = 1.0 / WSC

            for w in range(NWAVE):
                zw = zp.tile([128, TC, SEC], F16, tag="z", name=f"z{w}")

                # --- fill Z(w): tensor engine + scalar-engine copies ---
                if w <= NCHUNK - 1:  # L0 chunk w
                    halves = 2 if w == 0 else 1
                    half = TC // halves
                    for hv in range(halves):
                        tsl = slice(hv * half, (hv + 1) * half)
                        for o in range(8):
                            ps = pp.tile([128, half, BC], F32, tag="ps", name="ps")
                            for k in range(P0):
                                nc.tensor.matmul(
                                    ps[:],
                                    w0[k][:, :, o * 128:(o + 1) * 128],
                                    xt[k][:, :, w * TC + hv * half:w * TC + (hv + 1) * half, :],
                                    start=(k == 0), stop=(k == P0 - 1),
                                    perf_mode=DR,
                                )
                            nc.scalar.activation(
                                zw[:, tsl, o * 32:(o + 1) * 32], ps[:],
                                AF.Identity, bias=bias[:, o:o + 1], scale=ISC,
                            )
                if 2 <= w <= NCHUNK + 1:  # L1 chunk w-2, consumes Q8(w-2) L0 part
                    q8in = prev_q8[1]
                    halves = 2 if w == NCHUNK + 1 else 1
                    half = TC // halves
                    for hv in range(halves):
                        tsl = slice(hv * half, (hv + 1) * half)
                        for o in range(8):
                            ps = pp.tile([128, half, BC], F32, tag="ps", name="ps")
                            for k in range(P1):
                                nc.tensor.matmul(
                                    ps[:],
                                    w1[k][:, :, o * 128:(o + 1) * 128],
                                    q8in[:, tsl, k, :, :].rearrange(
                                        "p t i b -> p i t b"),
                                    start=(k == 0), stop=(k == P1 - 1),
                                    perf_mode=DR,
                                )
                            nc.scalar.activation(
                                zw[:, tsl, 256 + o * 32:256 + (o + 1) * 32], ps[:],
                                AF.Identity, bias=bias[:, 8 + o:9 + o], scale=ISC,
                            )
                if 3 <= w <= NCHUNK + 2:  # L2 chunk w-3, consumes Q8(w-1) L1 part
                    q8in = prev_q8[0]
                    nparts = 2
                    half = TC // nparts
                    for hv in range(nparts):
                        tsl = slice(hv * half, (hv + 1) * half)
                        ps = pp.tile([128, half, BC], F32, tag="ps2", name="ps2", bufs=2)
                        for k in range(P1):
                            nc.tensor.matmul(
                                ps[:],
                                w2[k][:],
                                q8in[:, tsl, 4 + k, :, :].rearrange(
                                    "p t i b -> p i t b"),
                                start=(k == 0), stop=(k == P1 - 1),
                                perf_mode=DR,
                            )
                        nc.scalar.activation(
                            zw[:, tsl, 512:SEC], ps[:, :, 0:32],
                            AF.Identity, bias=bias[:, 16:17], scale=ISC,
                        )

                # ACT L0-section convert of the PREVIOUS wave (after this
                # wave's psum->sbuf copies in ACT program order, so those
                # copies are not blocked behind the previous wave's scan)
                if pend_act is not None:
                    pq8f, psw = pend_act
                    nc.scalar.copy(pq8f[:, :, 0:256], psw[:, :, 0:256])
                    pend_act = None

                # --- scan wave w: 10 LIF steps over the active window ---
                lo, hi = _active_window(w)
                sw = sp.tile([128, TC, SEC], F16, tag="s", name=f"s{w}")
                for t in range(TC):
                    # state wst = 0.5 * v_post; u = v_pre; q = 0.5*(u<1)
                    u = up.tile([128, SEC], F16, tag="u", name="u")
                    nc.vector.tensor_tensor(
                        u[:, lo:hi], wst[:, lo:hi], zw[:, t, lo:hi], op=AL.add,
                    )
                    nc.vector.tensor_scalar(
                        sw[:, t, lo:hi], u[:, lo:hi], 1.0, 0.5,
                        op0=AL.is_lt, op1=AL.mult,
                    )
                    nc.vector.tensor_tensor(
                        wst[:, lo:hi], u[:, lo:hi], sw[:, t, lo:hi], op=AL.mult,
                    )

                # re-code spikes to fp8 for matmul consumption (exact: {0,0.5})
                # split across ACT (L0 half, delayed one wave) and DVE (L1
                # half, chain-resident right after the scan) to balance load
                q8 = qp.tile([128, TC, 8, 2, 32], F8, tag="q8", name=f"q8{w}")
                q8f = q8[:].rearrange("p t k i b -> p t (k i b)")
                if w <= NCHUNK - 1:      # L0 section feeds L1-mm at w+2
                    pend_act = (q8f, sw)
                if 2 <= w <= NCHUNK + 1:  # L1 section feeds L2-mm at w+1
                    nc.vector.tensor_scalar(
                        q8f[:, :, 256:512], sw[:, :, 256:512], 0.0, None,
                        op0=AL.add,
                    )

                if w >= 3:  # collect L2 spikes (chunk w-3)
                    nc.scalar.copy(
                        outq[:, (w - 3) * TC:(w - 2) * TC, :],
                        sw[0:CLS, :, 512:SEC],
                    )

                prev_q8 = [q8, prev_q8[0]]

            nc.sync.dma_start(QOUT[:], outq[:])

    nc.compile()
    return nc


def _get_nc():
    if "nc" not in _CACHE:
        _CACHE["nc"] = _build()
    return _CACHE["nc"]


def _get_runner():
    """Build (once) a cached jitted SPMD executable over the 8 cores."""
    if "runner" in _CACHE:
        return _CACHE["runner"]
    import jax
    from jax.sharding import Mesh, PartitionSpec
    from jax.experimental.shard_map import shard_map
    from concourse import bass2jax

    nc = _get_nc()
    bass2jax.install_neuronx_cc_hook()
    partition_name = (
        nc.partition_id_tensor.name if nc.partition_id_tensor else None
    )
    in_names, out_names, out_avals, zero_shapes = [], [], [], []
    for alloc in nc.m.functions[0].allocations:
        if not isinstance(alloc, mybir.MemoryLocationSet):
            continue
        name = alloc.memorylocations[0].name
        if alloc.kind == "ExternalInput":
            if name != partition_name:
                in_names.append(name)
        elif alloc.kind == "ExternalOutput":
            shape = tuple(alloc.tensor_shape)
            dtype = mybir.dt.np(alloc.dtype)
            out_names.append(name)
            out_avals.append(jax.core.ShapedArray(shape, dtype))
            zero_shapes.append((shape, dtype))
    n_params = len(in_names)
    all_in = in_names + out_names
    if partition_name is not None:
        all_in = all_in + [partition_name]

    def _body(*args):
        operands = list(args)
        if partition_name is not None:
            operands.append(bass2jax.partition_id_tensor())
        outs = bass2jax._bass_exec_p.bind(
            *operands,
            out_avals=tuple(out_avals),
            in_names=tuple(all_in),
            out_names=tuple(out_names),
            lowering_input_output_aliases=(),
            sim_require_finite=True,
            sim_require_nnan=True,
            nc=nc,
        )
        return tuple(outs)

    devices = jax.devices()[:NCORES]
    mesh = Mesh(np.asarray(devices), ("core",))
    donate = tuple(range(n_params, n_params + len(out_names)))
    sharded = jax.jit(
        shard_map(
            _body, mesh=mesh,
            in_specs=(PartitionSpec("core"),) * (n_params + len(out_names)),
            out_specs=(PartitionSpec("core"),) * len(out_names),
            check_rep=False,
        ),
        donate_argnums=donate, keep_unused=True,
    )

    def run(in_maps):
        concat_in = [
            np.concatenate([np.asarray(m[nm]) for m in in_maps], axis=0)
            for nm in in_names
        ]
        concat_zeros = [
            np.zeros((NCORES * sh[0], *sh[1:]), dt) for sh, dt in zero_shapes
        ]
        out_arrs = sharded(*concat_in, *concat_zeros)
        return [
            {
                nm: np.asarray(out_arrs[i]).reshape(NCORES, *out_avals[i].shape)[c]
                for i, nm in enumerate(out_names)
            }
            for c in range(NCORES)
        ]

    _CACHE["runner"] = run
    return run


def _to_f8(a):
    return np.clip(np.asarray(a, np.float32), -240.0, 240.0).astype(E4)


def _pack_dr_weights(wt, passes):
    """wt: [K, M] fp32 (K = passes*256) -> [passes, 128, 2, M] fp8."""
    K, M = wt.shape
    assert K == passes * 256
    return _to_f8(wt.reshape(passes, 2, 128, M).transpose(0, 2, 1, 3))


def kernel(x_tbf, W0, b0, W1, b1, W2, b2):
    global LAST_RESULT
    import os

    x = np.asarray(x_tbf, np.float32)
    W0 = np.asarray(W0, np.float32)
    W1 = np.asarray(W1, np.float32)
    W2 = np.asarray(W2, np.float32)
    b0 = np.asarray(b0, np.float32)
    b1 = np.asarray(b1, np.float32)
    b2 = np.asarray(b2, np.float32)

    # weights: fold the 0.5 (leak) scale and the q-code correction (s = 1-2q)
    # plus the fp8 range scale WSC (unscaled in the ACT psum->sbuf copy).
    w0t = np.zeros((P0 * 256, HID), np.float32)
    w0t[:FIN] = WSC * 0.5 * W0.T
    w1t = WSC * (-W1.T)                                   # [1024, 1024]
    w2t = np.zeros((HID, 128), np.float32)
    w2t[:, :CLS] = WSC * (-W2.T)

    w0t_r = _pack_dr_weights(w0t, P0)
    w1t_r = _pack_dr_weights(w1t, P1)
    w2t_r = _pack_dr_weights(w2t, P1)

    bias_arr = np.zeros((128, 17), np.float32)
    bias_arr[:, 0:8] = (0.5 * b0).reshape(8, 128).T
    b1e = 0.5 * (b1.astype(np.float64) + W1.astype(np.float64).sum(axis=1))
    bias_arr[:, 8:16] = b1e.astype(np.float32).reshape(8, 128).T
    b2e = 0.5 * (b2.astype(np.float64) + W2.astype(np.float64).sum(axis=1))
    bias_arr[:CLS, 16] = b2e.astype(np.float32)

    in_maps = []
    for c in range(NCORES):
        xs = x[:, c * BC:(c + 1) * BC, :]                 # [T, BC, FIN]
        xt = np.zeros((P0 * 256, T, BC), np.float32)
        xt[:FIN] = xs.transpose(2, 0, 1)
        xt_r = _to_f8(xt.reshape(P0, 2, 128, T, BC).transpose(0, 2, 1, 3, 4))
        in_maps.append({
            "XT": np.ascontiguousarray(xt_r),
            "W0T": w0t_r, "W1T": w1t_r, "W2T": w2t_r, "BIAS": bias_arr,
        })

    if os.environ.get("BASS_TRACE"):
        nc = _get_nc()
        LAST_RESULT = run_bass_kernel_spmd(
            nc, in_maps, list(range(NCORES)),
            trace=True,
            tmpdir=os.environ.get("BASS_TRACE_DIR"),
        )
        results = LAST_RESULT.results
    else:
        results = _get_runner()(in_maps)

    out = np.empty((T, B, CLS), np.float32)
    for c in range(NCORES):
        q = results[c]["QOUT"].astype(np.float32)  # [CLS, T, BC]
        out[:, c * BC:(c + 1) * BC, :] = (1.0 - 2.0 * q).transpose(1, 2, 0)
    return out


# revision 8
# speedup vs baseline: 1.1950x; 1.0686x over previous
"""Trainium2 Bass kernel for nn_BPSpikingNet (3-layer LIF spiking net).

Strategy (data-parallel over batch, 8 NeuronCores, zero collectives):
  - Host pre-packs x and all weights into fp8(e4m3) DoubleRow pair layouts;
    weights scaled by 64 (pow2, exact) to stay in fp8 normal range, unscaled
    via the ACT psum->sbuf copy's scale parameter. fp32 PSUM accumulation.
  - Matmuls run in MatmulPerfMode.DoubleRow: contraction 256 per pass
    (2x MACs/cycle vs fp16) -> L0: 3 passes, L1/L2: 4 passes per o-chunk.
  - Per core: stream T=100 in chunks of 10 steps. The three layers' LIF
    scans run as ONE concatenated DVE scan over [128, 544] fp16 state
    (L1 lags L0 by 2 waves, L2 by 3), 3 ops/step:
      u = w + z             [tensor_tensor]
      q = (u < 1) * 0.5     [tensor_scalar, fp16 out -> 2x DVE mode]
      w = u * q             [tensor_tensor]
  - Spikes are re-coded to fp8 for matmul consumption by a per-wave
    gpsimd cast-DMA (values {0,0.5} are exact in fp8); L1/L2 matmuls read
    the fp8 buffer through a strided (pair, t, batch) access pattern.
  - Output: q2 in [20, T, 32] per core; host maps s = 1 - 2q and gathers.
"""
import sys

import numpy as np

sys.path.insert(0, "/opt/trn_rl_repo")

import concourse.bass as bass
import concourse.tile as tile
from concourse import bacc, mybir
from concourse.bass_utils import run_bass_kernel_spmd
import ml_dtypes


def _install_ntff_shim():
    """Provide antenv.axon_hooks (missing in the trimmed image) so that
    trace=True NTFF profiling works when requested via BASS_TRACE."""
    try:
        import antenv.axon_hooks  # noqa: F401
        return
    except ImportError:
        pass
    try:
        import types

        import antenv

        mod = types.ModuleType("antenv.axon_hooks")
        holder = {"h": None}
        mod.set_axon_ntff_profile_hook = lambda h: holder.__setitem__("h", h)
        mod.get_axon_ntff_profile_hook = lambda: holder["h"]
        sys.modules["antenv.axon_hooks"] = mod
        antenv.axon_hooks = mod
        try:
            from trn_agent_boot.trn_boot import _ntff_profile_via_ctypes

            h = _ntff_profile_via_ctypes("/opt/axon/libaxon_pjrt.so")
            if h is not None:
                mod.set_axon_ntff_profile_hook(h)
        except Exception:
            pass
    except Exception:
        pass


_install_ntff_shim()

F32 = mybir.dt.float32
F16 = mybir.dt.float16
F8 = mybir.dt.float8e4
E4 = ml_dtypes.float8_e4m3
AL = mybir.AluOpType
AF = mybir.ActivationFunctionType
DR = mybir.MatmulPerfMode.DoubleRow

T, B, FIN, HID, CLS = 100, 256, 700, 1024, 20
NCORES = 8
BC = B // NCORES            # 32 batch rows per core
TC = 10                     # timesteps per chunk
NCHUNK = T // TC            # 10
NWAVE = NCHUNK + 4          # L1 lags L0 by 2; L2 half-wave shifted
P0 = 3                      # DoubleRow passes for layer 0 (K=768)
P1 = 4                      # DoubleRow passes for layers 1/2 (K=1024)
SEC = 544                   # 256 (L0 out) + 256 (L1 out) + 32 (L2 out)
WSC = 64.0                  # fp8 weight scale (pow2); ACT copies scale 1/64

_CACHE = {}
LAST_RESULT = None


def _active_window(w):
    """Column window [lo, hi) of the concat state active at wave w."""
    lo = 0 if w <= NCHUNK - 1 else (256 if w <= NCHUNK + 1 else 512)
    hi = 256 if w < 2 else (512 if w < 3 else SEC)
    return lo, hi


def _build():
    nc = bacc.Bacc(None, target_bir_lowering=False)
    XT = nc.declare_dram_parameter("XT", [P0, 128, 2, T, BC], F8, isOutput=False)
    W0T = nc.declare_dram_parameter("W0T", [P0, 128, 2, HID], F8, isOutput=False)
    W1T = nc.declare_dram_parameter("W1T", [P1, 128, 2, HID], F8, isOutput=False)
    W2T = nc.declare_dram_parameter("W2T", [P1, 128, 2, 128], F8, isOutput=False)
    BIAS = nc.declare_dram_parameter("BIAS", [128, 17], F32, isOutput=False)
    QOUT = nc.declare_dram_parameter("QOUT", [CLS, T, BC], F16, isOutput=True)

    with tile.TileContext(nc) as tc:
        with (
            tc.tile_pool(name="const", bufs=1) as cp,
            tc.tile_pool(name="zp", bufs=5) as zp,
            tc.tile_pool(name="sp", bufs=5) as sp,
            tc.tile_pool(name="qp", bufs=4) as qp,
            tc.tile_pool(name="up", bufs=4) as up,
            tc.tile_pool(name="pp", bufs=6, space=bass.MemorySpace.PSUM) as pp,
        ):
            xt = [cp.tile([128, 2, T, BC], F8, tag=f"xt{k}", name=f"xt{k}")
                  for k in range(P0)]
            w0 = [cp.tile([128, 2, HID], F8, tag=f"w0_{k}", name=f"w0_{k}")
                  for k in range(P0)]
            w1 = [cp.tile([128, 2, HID], F8, tag=f"w1_{k}", name=f"w1_{k}")
                  for k in range(P1)]
            w2 = [cp.tile([128, 2, 128], F8, tag=f"w2_{k}", name=f"w2_{k}")
                  for k in range(P1)]
            bias = cp.tile([128, 17], F32, tag="bias")
            wst = cp.tile([128, SEC], F16, tag="wst")
            outq = cp.tile([CLS, T, BC], F16, tag="outq")

            nc.scalar.dma_start(bias[:], BIAS[:])
            for k in range(P0):
                nc.sync.dma_start(w0[k][:], W0T[k])
                nc.sync.dma_start(xt[k][:, :, 0:TC, :], XT[k][:, :, 0:TC, :])
            for k in range(P0):
                nc.sync.dma_start(xt[k][:, :, TC:T, :], XT[k][:, :, TC:T, :])
            for k in range(P1):
                nc.sync.dma_start(w1[k][:], W1T[k])
                nc.sync.dma_start(w2[k][:], W2T[k])
            nc.gpsimd.memset(wst[:], 0.0)
            # warm the ACT activation-table during the DMA head so the first
            # psum->sbuf copy doesn't pay the ~2.7us table load
            warm = cp.tile([128, 1], F32, tag="warm")
            nc.vector.memset(warm[:], 0.0)
            nc.scalar.activation(warm[:], warm[:], AF.Identity,
                                 bias=bias[:, 0:1], scale=1.0)

            prev_q8 = [None, None]  # Q8(w-1), Q8(w-2) spike buffers (fp8)
            pend_act = None  # (q8f, sw) of prev wave: ACT L0-convert, delayed
                             # so it doesn't head-of-line block this wave's copies

            ISC = 1.0 / WSC

            for w in range(NWAVE):
                zw = zp.tile([128, TC, SEC], F16, tag="z", name=f"z{w}")

                # --- fill Z(w): tensor engine + scalar-engine copies ---
                if w <= NCHUNK - 1:  # L0 chunk w
                    halves = 2 if w == 0 else 1
                    half = TC // halves
                    for hv in range(halves):
                        tsl = slice(hv * half, (hv + 1) * half)
                        for o in range(8):
                            ps = pp.tile([128, half, BC], F32, tag="ps", name="ps")
                            for k in range(P0):
                                nc.tensor.matmul(
                                    ps[:],
                                    w0[k][:, :, o * 128:(o + 1) * 128],
                                    xt[k][:, :, w * TC + hv * half:w * TC + (hv + 1) * half, :],
                                    start=(k == 0), stop=(k == P0 - 1),
                                    perf_mode=DR,
                                )
                            nc.scalar.activation(
                                zw[:, tsl, o * 32:(o + 1) * 32], ps[:],
                                AF.Identity, bias=bias[:, o:o + 1], scale=ISC,
                            )
                if 2 <= w <= NCHUNK + 1:  # L1 chunk w-2, consumes Q8(w-2) L0 part
                    q8in = prev_q8[1]
                    halves = 2 if w == NCHUNK + 1 else 1
                    half = TC // halves
                    for hv in range(halves):
                        tsl = slice(hv * half, (hv + 1) * half)
                        for o in range(8):
                            ps = pp.tile([128, half, BC], F32, tag="ps", name="ps")
                            for k in range(P1):
                                nc.tensor.matmul(
                                    ps[:],
                                    w1[k][:, :, o * 128:(o + 1) * 128],
                                    q8in[:, tsl, k, :, :].rearrange(
                                        "p t i b -> p i t b"),
                                    start=(k == 0), stop=(k == P1 - 1),
                                    perf_mode=DR,
                                )
                            nc.scalar.activation(
                                zw[:, tsl, 256 + o * 32:256 + (o + 1) * 32], ps[:],
                                AF.Identity, bias=bias[:, 8 + o:9 + o], scale=ISC,
                            )
                if 3 <= w <= NCHUNK + 3:
                    # L2, half-wave shifted: slot t of wave w holds L2 step
                    # g = 10w + t - 35, so the scan start never waits on the
                    # freshest q8-L1 convert (5 slots of slack).
                    half = TC // 2
                    parts = []
                    c1 = w - 4   # second half of chunk w-4 -> slots 0:5
                    if 0 <= c1 <= NCHUNK - 1:
                        parts.append((slice(half, TC), slice(0, half), prev_q8[1]))
                    c0 = w - 3   # first half of chunk w-3 -> slots 5:10
                    if 0 <= c0 <= NCHUNK - 1:
                        parts.append((slice(0, half), slice(half, TC), prev_q8[0]))
                    for src_t, dst_t, q8in in parts:
                        ps = pp.tile([128, half, BC], F32, tag="ps2", name="ps2", bufs=2)
                        for k in range(P1):
                            nc.tensor.matmul(
                                ps[:],
                                w2[k][:],
                                q8in[:, src_t, 4 + k, :, :].rearrange(
                                    "p t i b -> p i t b"),
                                start=(k == 0), stop=(k == P1 - 1),
                                perf_mode=DR,
                            )
                        nc.scalar.activation(
                            zw[:, dst_t, 512:SEC], ps[:, :, 0:32],
                            AF.Identity, bias=bias[:, 16:17], scale=ISC,
                        )
                    # zero-fill the undefined edge slots (state stays 0 there)
                    if w == 3:
                        nc.gpsimd.memset(zw[:, 0:half, 512:SEC], 0.0)
                    if w == NCHUNK + 3:
                        nc.gpsimd.memset(zw[:, half:TC, 512:SEC], 0.0)

                # ACT L0-section convert of the PREVIOUS wave (after this
                # wave's psum->sbuf copies in ACT program order, so those
                # copies are not blocked behind the previous wave's scan)
                if pend_act is not None:
                    pq8f, psw = pend_act
                    nc.scalar.copy(pq8f[:, :, 0:256], psw[:, :, 0:256])
                    pend_act = None

                # --- scan wave w: 10 LIF steps over the active window ---
                lo, hi = _active_window(w)
                sw = sp.tile([128, TC, SEC], F16, tag="s", name=f"s{w}")
                for t in range(TC):
                    # state wst = 0.5 * v_post; u = v_pre; q = 0.5*(u<1)
                    u = up.tile([128, SEC], F16, tag="u", name="u")
                    nc.vector.tensor_tensor(
                        u[:, lo:hi], wst[:, lo:hi], zw[:, t, lo:hi], op=AL.add,
                    )
                    nc.vector.tensor_scalar(
                        sw[:, t, lo:hi], u[:, lo:hi], 1.0, 0.5,
                        op0=AL.is_lt, op1=AL.mult,
                    )
                    nc.vector.tensor_tensor(
                        wst[:, lo:hi], u[:, lo:hi], sw[:, t, lo:hi], op=AL.mult,
                    )

                # re-code spikes to fp8 for matmul consumption (exact: {0,0.5})
                # split across ACT (L0 half, delayed one wave) and DVE (L1
                # half, chain-resident right after the scan) to balance load
                q8 = qp.tile([128, TC, 8, 2, 32], F8, tag="q8", name=f"q8{w}")
                q8f = q8[:].rearrange("p t k i b -> p t (k i b)")
                if w <= NCHUNK - 1:      # L0 section feeds L1-mm at w+2
                    pend_act = (q8f, sw)
                if 2 <= w <= NCHUNK + 1:  # L1 section feeds L2-mm at w+1
                    nc.vector.tensor_scalar(
                        q8f[:, :, 256:512], sw[:, :, 256:512], 0.0, None,
                        op0=AL.add,
                    )

                if w >= 3:  # collect L2 spikes (g = 10w + t - 35)
                    half = TC // 2
                    if w == 3:
                        nc.scalar.copy(outq[:, 0:half, :],
                                       sw[0:CLS, half:TC, 512:SEC])
                    elif w == NCHUNK + 3:
                        nc.scalar.copy(outq[:, T - half:T, :],
                                       sw[0:CLS, 0:half, 512:SEC])
                    else:
                        g0 = (w - 4) * TC + half
                        nc.scalar.copy(outq[:, g0:g0 + TC, :],
                                       sw[0:CLS, :, 512:SEC])

                prev_q8 = [q8, prev_q8[0]]

            nc.sync.dma_start(QOUT[:], outq[:])

    nc.compile()
    return nc


def _get_nc():
    if "nc" not in _CACHE:
        _CACHE["nc"] = _build()
    return _CACHE["nc"]


def _get_runner():
    """Build (once) a cached jitted SPMD executable over the 8 cores."""
    if "runner" in _CACHE:
        return _CACHE["runner"]
    import jax
    from jax.sharding import Mesh, PartitionSpec
    from jax.experimental.shard_map import shard_map
    from concourse import bass2jax

    nc = _get_nc()
    bass2jax.install_neuronx_cc_hook()
    partition_name = (
        nc.partition_id_tensor.name if nc.partition_id_tensor else None
    )
    in_names, out_names, out_avals, zero_shapes = [], [], [], []
    for alloc in nc.m.functions[0].allocations:
        if not isinstance(alloc, mybir.MemoryLocationSet):
            continue
        name = alloc.memorylocations[0].name
        if alloc.kind == "ExternalInput":
            if name != partition_name:
                in_names.append(name)
        elif alloc.kind == "ExternalOutput":
            shape = tuple(alloc.tensor_shape)
            dtype = mybir.dt.np(alloc.dtype)
            out_names.append(name)
            out_avals.append(jax.core.ShapedArray(shape, dtype))
            zero_shapes.append((shape, dtype))
    n_params = len(in_names)
    all_in = in_names + out_names
    if partition_name is not None:
        all_in = all_in + [partition_name]

    def _body(*args):
        operands = list(args)
        if partition_name is not None:
            operands.append(bass2jax.partition_id_tensor())
        outs = bass2jax._bass_exec_p.bind(
            *operands,
            out_avals=tuple(out_avals),
            in_names=tuple(all_in),
            out_names=tuple(out_names),
            lowering_input_output_aliases=(),
            sim_require_finite=True,
            sim_require_nnan=True,
            nc=nc,
        )
        return tuple(outs)

    devices = jax.devices()[:NCORES]
    mesh = Mesh(np.asarray(devices), ("core",))
    donate = tuple(range(n_params, n_params + len(out_names)))
    sharded = jax.jit(
        shard_map(
            _body, mesh=mesh,
            in_specs=(PartitionSpec("core"),) * (n_params + len(out_names)),
            out_specs=(PartitionSpec("core"),) * len(out_names),
            check_rep=False,
        ),
        donate_argnums=donate, keep_unused=True,
    )

    def run(in_maps):
        concat_in = [
            np.concatenate([np.asarray(m[nm]) for m in in_maps], axis=0)
            for nm in in_names
        ]
        concat_zeros = [
            np.zeros((NCORES * sh[0], *sh[1:]), dt) for sh, dt in zero_shapes
        ]
        out_arrs = sharded(*concat_in, *concat_zeros)
        return [
            {
                nm: np.asarray(out_arrs[i]).reshape(NCORES, *out_avals[i].shape)[c]
                for i, nm in enumerate(out_names)
            }
            for c in range(NCORES)
        ]

    _CACHE["runner"] = run
    return run


def _to_f8(a):
    return np.clip(np.asarray(a, np.float32), -240.0, 240.0).astype(E4)


def _pack_dr_weights(wt, passes):
    """wt: [K, M] fp32 (K = passes*256) -> [passes, 128, 2, M] fp8."""
    K, M = wt.shape
    assert K == passes * 256
    return _to_f8(wt.reshape(passes, 2, 128, M).transpose(0, 2, 1, 3))


def kernel(x_tbf, W0, b0, W1, b1, W2, b2):
    global LAST_RESULT
    import os

    x = np.asarray(x_tbf, np.float32)
    W0 = np.asarray(W0, np.float32)
    W1 = np.asarray(W1, np.float32)
    W2 = np.asarray(W2, np.float32)
    b0 = np.asarray(b0, np.float32)
    b1 = np.asarray(b1, np.float32)
    b2 = np.asarray(b2, np.float32)

    # weights: fold the 0.5 (leak) scale and the q-code correction (s = 1-2q)
    # plus the fp8 range scale WSC (unscaled in the ACT psum->sbuf copy).
    w0t = np.zeros((P0 * 256, HID), np.float32)
    w0t[:FIN] = WSC * 0.5 * W0.T
    w1t = WSC * (-W1.T)                                   # [1024, 1024]
    w2t = np.zeros((HID, 128), np.float32)
    w2t[:, :CLS] = WSC * (-W2.T)

    w0t_r = _pack_dr_weights(w0t, P0)
    w1t_r = _pack_dr_weights(w1t, P1)
    w2t_r = _pack_dr_weights(w2t, P1)

    bias_arr = np.zeros((128, 17), np.float32)
    bias_arr[:, 0:8] = (0.5 * b0).reshape(8, 128).T
    b1e = 0.5 * (b1.astype(np.float64) + W1.astype(np.float64).sum(axis=1))
    bias_arr[:, 8:16] = b1e.astype(np.float32).reshape(8, 128).T
    b2e = 0.5 * (b2.astype(np.float64) + W2.astype(np.float64).sum(axis=1))
    bias_arr[:CLS, 16] = b2e.astype(np.float32)

    in_maps = []
    for c in range(NCORES):
        xs = x[:, c * BC:(c + 1) * BC, :]                 # [T, BC, FIN]
        xt = np.zeros((P0 * 256, T, BC), np.float32)
        xt[:FIN] = xs.transpose(2, 0, 1)
        xt_r = _to_f8(xt.reshape(P0, 2, 128, T, BC).transpose(0, 2, 1, 3, 4))
        in_maps.append({
            "XT": np.ascontiguousarray(xt_r),
            "W0T": w0t_r, "W1T": w1t_r, "W2T": w2t_r, "BIAS": bias_arr,
        })

    if os.environ.get("BASS_TRACE"):
        nc = _get_nc()
        LAST_RESULT = run_bass_kernel_spmd(
            nc, in_maps, list(range(NCORES)),
            trace=True,
            tmpdir=os.environ.get("BASS_TRACE_DIR"),
        )
        results = LAST_RESULT.results
    else:
        results = _get_runner()(in_maps)

    out = np.empty((T, B, CLS), np.float32)
    for c in range(NCORES):
        q = results[c]["QOUT"].astype(np.float32)  # [CLS, T, BC]
        out[:, c * BC:(c + 1) * BC, :] = (1.0 - 2.0 * q).transpose(1, 2, 0)
    return out
